# revision 4
# baseline (speedup 1.0000x reference)
"""TRN2 Bass kernel for nn_AdapterModule (ROI-align + conv1x1 + ReLU + avgpool + MLP).

Sharding: core = 2*b + half; 512 y-sorted vertices of batch b per core; each
core reads a 152-row y-band of its batch's features (bf16, host-cast).

Pipeline per core:
  1. conv1x1 (channels 256->128, weights pre-scaled 1/49) on the band,
     f-stationary matmuls -> pos-major bf16 table in DRAM [38912, 128].
  2. indirect-DMA gather: one 8-pixel row-run (2 KB) per (vertex, patch row),
     35 rounds x 15 vertices -> SBUF [partition=(v,di), free=(dj,o)].
  3. X-lerp on DVE (fused scalar_tensor_tensor, per-partition lx).
  4. Y-lerp + conv bias on PE: block-diag Ay matmul contracting partitions;
     a spare partition row injects the bias.
  5. ACT ReLU (PSUM->SBUF bf16); pool = ones-matmul over i (PE) + DVE reduce
     over j; PE transpose -> pooled [128 ch, vertex].
  6. MLP 128->256->128->4 on PE (3 heads fused into one [4,128] matmul).

The 49 ROI samples of a vertex share one fractional offset (sample spacing
== 1 px == bin size), so they form an 8x8 patch with a separable 2x2 stencil;
for vertices in [4, 252] clipping/validity never trigger.
"""

import numpy as np

ROI = 7
B, C, H, W, N = 4, 256, 256, 256, 1024
NV_CORE = 512       # vertices per core
VR = 15             # vertices per round
NR = 35             # rounds (35*15 = 525 >= 512)
VPAD = NR * VR      # 525
BAND = 152          # band rows per core
NP = BAND * 256     # band positions
Y0 = [0, 104]       # band start row for half 0 / 1
NG4 = NP // 512     # conv groups of 4 pos-tiles


def _bf16(x):
    import ml_dtypes
    return np.asarray(x, dtype=np.float32).astype(ml_dtypes.bfloat16)


def _host_prep(features, vertices, Wc, bc, W1, b1, W2, b2, Wa, ba, Wr, br, Ww, bw):
    f32 = np.float32
    feats_bf = _bf16(np.asarray(features, f32).reshape(B, C, H * W))

    wc49 = np.asarray(Wc, f32) / f32(49.0)                # [128, 256]
    bc49 = np.asarray(bc, f32) / f32(49.0)                # [128]
    wct = np.concatenate([wc49[:, 0:128].T, wc49[:, 128:256].T], axis=1)
    bcrow = np.tile(bc49[None, :], (8, 7)).astype(f32)    # [8, 896]

    onesb = np.zeros((105, VR), f32)
    for k in range(VR):
        onesb[k * 7:(k + 1) * 7, k] = 1.0

    w1 = np.asarray(W1, f32)
    w1t = np.concatenate([w1[:, 0:128][0:0]] if False else
                         [w1[0:128, :].T, w1[128:256, :].T], axis=1)  # [128,256]
    b1a = np.asarray(b1, f32).reshape(2, 128).T           # [128, 2]
    w2 = np.asarray(W2, f32)
    w2t = np.concatenate([w2[:, 0:128].T, w2[:, 128:256].T], axis=1)  # [128,256]
    b2a = np.asarray(b2, f32).reshape(128, 1)
    wh = np.concatenate([np.asarray(Wa, f32), np.asarray(Wr, f32),
                         np.asarray(Ww, f32)], axis=0)
    wht = np.ascontiguousarray(wh.T)                      # [128, 4]
    bha = np.concatenate([np.asarray(ba, f32), np.asarray(br, f32),
                          np.asarray(bw, f32)]).reshape(4, 1)
    ident = np.eye(128, dtype=f32)

    in_maps = []
    perms = []
    for b in range(B):
        v = np.asarray(vertices[b], f32)
        order = np.argsort(v[:, 1], kind="stable")
        perms.append(order)
        for half in range(2):
            sel = order[half * NV_CORE:(half + 1) * NV_CORE]
            vx = v[sel, 0].astype(f32)
            vy = v[sel, 1].astype(f32)
            sx = ((vx - f32(4.0)) + f32(0.5)).astype(f32)  # ref: start + offs[0]
            sy = ((vy - f32(4.0)) + f32(0.5)).astype(f32)
            fx = np.floor(sx).astype(np.int32)
            fy = np.floor(sy).astype(np.int32)
            lx = (sx - fx.astype(f32)).astype(f32)
            ly = (sy - fy.astype(f32)).astype(f32)
            fyb = fy - Y0[half]
            assert fyb.min() >= 0 and fyb.max() + 7 < BAND, (
                f"band overflow: fyb in [{fyb.min()}, {fyb.max()}]")

            pad = np.concatenate([np.arange(NV_CORE),
                                  np.full(VPAD - NV_CORE, NV_CORE - 1)])
            fxp, fybp = fx[pad], fyb[pad]
            lxp, lyp = lx[pad], ly[pad]

            idx = np.zeros((128, NR), np.int32)
            lxb = np.zeros((128, NR), f32)
            ayb = np.zeros((128, NR * 105), f32)
            for r in range(NR):
                blk = ayb[:, r * 105:(r + 1) * 105]
                for k in range(VR):
                    vcol = r * VR + k
                    for di in range(8):
                        idx[k * 8 + di, r] = (int(fybp[vcol]) + di) * 256 + int(fxp[vcol])
                    lxb[k * 8:(k + 1) * 8, r] = lxp[vcol]
                    lyv = lyp[vcol]
                    for i in range(7):
                        blk[k * 8 + i, k * 7 + i] += f32(1.0) - lyv
                        blk[k * 8 + i + 1, k * 7 + i] += lyv
                blk[120, :] = 1.0

            fband = feats_bf[b][:, Y0[half] * 256:(Y0[half] + BAND) * 256]

            in_maps.append({
                "fband": np.ascontiguousarray(fband),
                "wct": _bf16(wct),
                "ayb": _bf16(ayb),
                "idx": idx,
                "lxb": lxb,
                "bcrow": _bf16(bcrow),
                "onesb": onesb,
                "ident": ident,
                "w1t": w1t.astype(f32),
                "b1a": b1a,
                "w2t": w2t.astype(f32),
                "b2a": b2a,
                "wht": wht.astype(f32),
                "bha": bha,
            })
    return in_maps, perms


def _build_graph():
    import concourse.bass as bass
    import concourse.mybir as mybir
    from concourse import bacc
    from concourse.tile import TileContext

    nc = bacc.Bacc("TRN2", num_devices=8)
    dt = mybir.dt
    f_ = dt.float32
    bf = dt.bfloat16

    fband = nc.dram_tensor("fband", [256, NP], bf, kind="ExternalInput")
    wct = nc.dram_tensor("wct", [128, 256], bf, kind="ExternalInput")
    ayb = nc.dram_tensor("ayb", [128, NR * 105], bf, kind="ExternalInput")
    idx = nc.dram_tensor("idx", [128, NR], dt.int32, kind="ExternalInput")
    lxb = nc.dram_tensor("lxb", [128, NR], f_, kind="ExternalInput")
    bcrow = nc.dram_tensor("bcrow", [8, 896], bf, kind="ExternalInput")
    onesb = nc.dram_tensor("onesb", [105, VR], f_, kind="ExternalInput")
    ident = nc.dram_tensor("ident", [128, 128], f_, kind="ExternalInput")
    w1t = nc.dram_tensor("w1t", [128, 256], f_, kind="ExternalInput")
    b1a = nc.dram_tensor("b1a", [128, 2], f_, kind="ExternalInput")
    w2t = nc.dram_tensor("w2t", [128, 256], f_, kind="ExternalInput")
    b2a = nc.dram_tensor("b2a", [128, 1], f_, kind="ExternalInput")
    wht = nc.dram_tensor("wht", [128, 4], f_, kind="ExternalInput")
    bha = nc.dram_tensor("bha", [4, 1], f_, kind="ExternalInput")
    out = nc.dram_tensor("out", [4, VPAD], f_, kind="ExternalOutput")
    table = nc.dram_tensor("gtab", [NP, 128], bf, kind="Internal")

    RELU = mybir.ActivationFunctionType.Relu
    IDEN = mybir.ActivationFunctionType.Identity
    ADD = mybir.AluOpType.add
    SUB = mybir.AluOpType.subtract
    MULT = mybir.AluOpType.mult

    with TileContext(nc) as tc:
        # ---------------- Phase 1: conv1x1 band -> pos-major table --------
        with (
            tc.tile_pool(name="cconst", bufs=1) as cpool,
            tc.tile_pool(name="fload", bufs=3) as fpool,
            tc.tile_pool(name="gstage", bufs=3) as gpool,
            tc.tile_pool(name="cpsum", bufs=2, space="PSUM") as cpsum,
        ):
            wct_sb = cpool.tile([128, 256], bf, tag="wct")
            nc.sync.dma_start(wct_sb[:, :], wct[:, :])
            for g in range(NG4):
                ft0 = fpool.tile([128, 512], bf, tag="f0")
                ft1 = fpool.tile([128, 512], bf, tag="f1")
                nc.sync.dma_start(ft0[:, :], fband[0:128, g * 512:(g + 1) * 512])
                nc.sync.dma_start(ft1[:, :], fband[128:256, g * 512:(g + 1) * 512])
                ps = cpsum.tile([128, 512], f_, tag="cps")
                for t in range(4):
                    nc.tensor.matmul(ps[:, t * 128:(t + 1) * 128],
                                     ft0[:, t * 128:(t + 1) * 128],
                                     wct_sb[:, 0:128], start=True, stop=False)
                    nc.tensor.matmul(ps[:, t * 128:(t + 1) * 128],
                                     ft1[:, t * 128:(t + 1) * 128],
                                     wct_sb[:, 128:256], start=False, stop=True)
                gt = gpool.tile([128, 512], bf, tag="gt")
                if g % 2 == 0:
                    nc.vector.tensor_copy(gt[:, :], ps[:, :])
                else:
                    nc.scalar.copy(gt[:, :], ps[:, :])
                for t in range(4):
                    nc.sync.dma_start(
                        table[g * 512 + t * 128:g * 512 + (t + 1) * 128, :],
                        gt[:, t * 128:(t + 1) * 128])

        tc.strict_bb_all_engine_barrier()

        # ---------------- Phase 2: gather + interp + pool + MLP -----------
        with (
            tc.tile_pool(name="gconst", bufs=1) as ipool,
            tc.tile_pool(name="po", bufs=4) as popool,
            tc.tile_pool(name="interp", bufs=3) as xpool,
            tc.tile_pool(name="mpsum", bufs=2, space="PSUM") as mpsum,
            tc.tile_pool(name="ppsum", bufs=2, space="PSUM") as ppsum,
        ):
            idx_sb = ipool.tile([128, NR], dt.int32, tag="idx")
            nc.sync.dma_start(idx_sb[:, :], idx[:, :])
            lxb_sb = ipool.tile([128, NR], f_, tag="lxb")
            nc.sync.dma_start(lxb_sb[:, :], lxb[:, :])
            ayb_sb = ipool.tile([128, NR * 105], bf, tag="ayb")
            nc.sync.dma_start(ayb_sb[:, :], ayb[:, :])
            onesb_sb = ipool.tile([105, VR], f_, tag="onesb")
            nc.sync.dma_start(onesb_sb[:, :], onesb[:, :])
            ident_sb = ipool.tile([128, 128], f_, tag="ident")
            nc.sync.dma_start(ident_sb[:, :], ident[:, :])
            pooled = ipool.tile([128, VPAD], f_, tag="pooled")

            for r in range(NR):
                po = popool.tile([128, 1024], bf, tag="po")
                nc.gpsimd.indirect_dma_start(
                    out=po[0:120, :],
                    out_offset=None,
                    in_=table[:, :],
                    in_offset=bass.IndirectOffsetOnAxis(
                        ap=idx_sb[0:120, r:r + 1], axis=0),
                )
                d = xpool.tile([128, 896], bf, tag="d")
                nc.vector.tensor_tensor(
                    out=d[0:120, :], in0=po[0:120, 128:1024],
                    in1=po[0:120, 0:896], op=SUB)
                tx = xpool.tile([128, 896], bf, tag="tx")
                nc.vector.scalar_tensor_tensor(
                    out=tx[0:120, :], in0=d[0:120, :],
                    scalar=lxb_sb[0:120, r:r + 1],
                    in1=po[0:120, 0:896], op0=MULT, op1=ADD)
                nc.sync.dma_start(tx[120:128, :], bcrow[:, :])

                comp = mpsum.tile([105, 896], f_, tag="comp")
                nc.tensor.matmul(comp[:, 0:512],
                                 ayb_sb[:, r * 105:(r + 1) * 105],
                                 tx[:, 0:512], start=True, stop=True)
                nc.tensor.matmul(comp[:, 512:896],
                                 ayb_sb[:, r * 105:(r + 1) * 105],
                                 tx[:, 512:896], start=True, stop=True)
                rl = xpool.tile([105, 896], f_, tag="rl")
                nc.scalar.activation(rl[:, :], comp[:, :], RELU)

                pj = ppsum.tile([15, 896], f_, tag="pj")
                nc.tensor.matmul(pj[:, 0:512], onesb_sb[:, :],
                                 rl[:, 0:512], start=True, stop=True)
                nc.tensor.matmul(pj[:, 512:896], onesb_sb[:, :],
                                 rl[:, 512:896], start=True, stop=True)
                pv = xpool.tile([15, 128], f_, tag="pv")
                nc.vector.tensor_reduce(
                    pv[:, :],
                    pj[:, :].rearrange("p (j o) -> p o j", o=128),
                    axis=mybir.AxisListType.X, op=ADD)
                pt = mpsum.tile([128, VR], f_, tag="comp")
                nc.tensor.transpose(pt[:, :], pv[:, :], ident_sb[0:15, 0:15])
                if r % 2 == 0:
                    nc.vector.tensor_copy(pooled[:, r * VR:(r + 1) * VR], pt[:, :])
                else:
                    nc.scalar.copy(pooled[:, r * VR:(r + 1) * VR], pt[:, :])

            # ---------------- Phase 3: MLP --------------------------------
            w1_sb = ipool.tile([128, 256], f_, tag="w1t")
            nc.sync.dma_start(w1_sb[:, :], w1t[:, :])
            b1_sb = ipool.tile([128, 2], f_, tag="b1a")
            nc.sync.dma_start(b1_sb[:, :], b1a[:, :])
            w2_sb = ipool.tile([128, 256], f_, tag="w2t")
            nc.sync.dma_start(w2_sb[:, :], w2t[:, :])
            b2_sb = ipool.tile([128, 1], f_, tag="b2a")
            nc.sync.dma_start(b2_sb[:, :], b2a[:, :])
            wh_sb = ipool.tile([128, 4], f_, tag="wht")
            nc.sync.dma_start(wh_sb[:, :], wht[:, :])
            bh_sb = ipool.tile([4, 1], f_, tag="bha")
            nc.sync.dma_start(bh_sb[:, :], bha[:, :])

            NSPL = [(0, 512), (512, VPAD)]
            h1 = ipool.tile([128, 2 * VPAD], f_, tag="h1")
            for hh in range(2):
                hp = mpsum.tile([128, VPAD], f_, tag="comp")
                for (a, e) in NSPL:
                    nc.tensor.matmul(hp[:, a:e],
                                     w1_sb[:, hh * 128:(hh + 1) * 128],
                                     pooled[:, a:e], start=True, stop=True)
                nc.scalar.activation(h1[:, hh * VPAD:(hh + 1) * VPAD], hp[:, :],
                                     RELU, bias=b1_sb[:, hh:hh + 1])
            h2 = ipool.tile([128, VPAD], f_, tag="h2")
            hp2 = mpsum.tile([128, VPAD], f_, tag="comp")
            for (a, e) in NSPL:
                nc.tensor.matmul(hp2[:, a:e], w2_sb[:, 0:128],
                                 h1[:, a:e], start=True, stop=False)
                nc.tensor.matmul(hp2[:, a:e], w2_sb[:, 128:256],
                                 h1[:, VPAD + a:VPAD + e], start=False, stop=True)
            nc.scalar.activation(h2[:, :], hp2[:, :], RELU, bias=b2_sb[:, 0:1])
            ho = ipool.tile([4, VPAD], f_, tag="ho")
            hpo = ppsum.tile([4, VPAD], f_, tag="pj")
            for (a, e) in NSPL:
                nc.tensor.matmul(hpo[:, a:e], wh_sb[:, :], h2[:, a:e],
                                 start=True, stop=True)
            nc.scalar.activation(ho[:, :], hpo[:, :], IDEN, bias=bh_sb[:, 0:1])
            nc.sync.dma_start(out[:, :], ho[:, :])

    nc.compile()
    return nc


_GRAPH_CACHE = {}


def kernel(features, vertices, Wc, bc, W1, b1, W2, b2, Wa, ba, Wr, br, Ww, bw):
    import sys
    if '/opt/trn_rl_repo' not in sys.path:
        sys.path.insert(0, '/opt/trn_rl_repo')
    from concourse import bass_utils

    in_maps, perms = _host_prep(features, vertices, Wc, bc, W1, b1, W2, b2,
                                Wa, ba, Wr, br, Ww, bw)
    if "nc" not in _GRAPH_CACHE:
        _GRAPH_CACHE["nc"] = _build_graph()
    nc = _GRAPH_CACHE["nc"]
    import os
    trace = bool(int(os.environ.get("KERNEL_TRACE", "0")))
    res = bass_utils.run_bass_kernel_spmd(nc, in_maps, core_ids=list(range(8)),
                                          trace=trace)
    kernel.last_results = res
    kernel.last_in_maps = in_maps

    f32 = np.float32
    d_angle = np.zeros((B, N, 2), f32)
    d_radius = np.zeros((B, N, 1), f32)
    d_width = np.zeros((B, N, 1), f32)
    for b in range(B):
        order = perms[b]
        for half in range(2):
            o = np.asarray(res.results[2 * b + half]["out"], f32)  # [4, 525]
            sel = order[half * NV_CORE:(half + 1) * NV_CORE]
            d_angle[b, sel, 0] = o[0, :NV_CORE]
            d_angle[b, sel, 1] = o[1, :NV_CORE]
            d_radius[b, sel, 0] = o[2, :NV_CORE]
            d_width[b, sel, 0] = o[3, :NV_CORE]
    return d_angle, d_radius, d_width


# revision 5
# speedup vs baseline: 1.3087x; 1.3087x over previous
"""TRN2 Bass kernel for nn_AdapterModule (ROI-align + conv1x1 + ReLU + avgpool + MLP).

Sharding: core = 2*b + half; 512 y-sorted vertices of batch b per core; each
core reads a 152-row y-band of its batch's features (bf16, host-cast).

Pipeline per core:
  1. conv1x1 (channels 256->128, weights pre-scaled 1/49) on the band,
     f-stationary matmuls -> pos-major bf16 table in DRAM [38912, 128].
  2. indirect-DMA gather: one 8-pixel row-run (2 KB) per (vertex, patch row),
     35 rounds x 15 vertices -> SBUF [partition=(v,di), free=(dj,o)].
  3. X-lerp on DVE (fused scalar_tensor_tensor, per-partition lx).
  4. Y-lerp + conv bias on PE: block-diag Ay matmul contracting partitions;
     a spare partition row injects the bias.
  5. ACT ReLU (PSUM->SBUF bf16); pool = ones-matmul over i (PE) + DVE reduce
     over j; PE transpose -> pooled [128 ch, vertex].
  6. MLP 128->256->128->4 on PE (3 heads fused into one [4,128] matmul).

The 49 ROI samples of a vertex share one fractional offset (sample spacing
== 1 px == bin size), so they form an 8x8 patch with a separable 2x2 stencil;
for vertices in [4, 252] clipping/validity never trigger.
"""

import numpy as np

ROI = 7
B, C, H, W, N = 4, 256, 256, 256, 1024
NV_CORE = 512       # vertices per core
VR = 15             # vertices per round
NR = 35             # rounds (35*15 = 525 >= 512)
VPAD = NR * VR      # 525
BAND = 152          # band rows per core
NP = BAND * 256     # band positions
Y0 = [0, 104]       # band start row for half 0 / 1
NG4 = NP // 512     # conv groups of 4 pos-tiles


def _bf16(x):
    import ml_dtypes
    return np.asarray(x, dtype=np.float32).astype(ml_dtypes.bfloat16)


def _host_prep(features, vertices, Wc, bc, W1, b1, W2, b2, Wa, ba, Wr, br, Ww, bw):
    f32 = np.float32
    feats_bf = _bf16(np.asarray(features, f32).reshape(B, C, H * W))

    wc49 = np.asarray(Wc, f32) / f32(49.0)                # [128, 256]
    bc49 = np.asarray(bc, f32) / f32(49.0)                # [128]
    wct = np.concatenate([wc49[:, 0:128].T, wc49[:, 128:256].T], axis=1)
    bcrow = np.tile(bc49[None, :], (8, 7)).astype(f32)    # [8, 896]

    onesb = np.zeros((105, VR), f32)
    for k in range(VR):
        onesb[k * 7:(k + 1) * 7, k] = 1.0

    w1 = np.asarray(W1, f32)
    w1t = np.concatenate([w1[:, 0:128][0:0]] if False else
                         [w1[0:128, :].T, w1[128:256, :].T], axis=1)  # [128,256]
    b1a = np.asarray(b1, f32).reshape(2, 128).T           # [128, 2]
    w2 = np.asarray(W2, f32)
    w2t = np.concatenate([w2[:, 0:128].T, w2[:, 128:256].T], axis=1)  # [128,256]
    b2a = np.asarray(b2, f32).reshape(128, 1)
    wh = np.concatenate([np.asarray(Wa, f32), np.asarray(Wr, f32),
                         np.asarray(Ww, f32)], axis=0)
    wht = np.ascontiguousarray(wh.T)                      # [128, 4]
    bha = np.concatenate([np.asarray(ba, f32), np.asarray(br, f32),
                          np.asarray(bw, f32)]).reshape(4, 1)
    ident = np.eye(128, dtype=f32)

    in_maps = []
    perms = []
    for b in range(B):
        v = np.asarray(vertices[b], f32)
        order = np.argsort(v[:, 1], kind="stable")
        perms.append(order)
        for half in range(2):
            sel = order[half * NV_CORE:(half + 1) * NV_CORE]
            vx = v[sel, 0].astype(f32)
            vy = v[sel, 1].astype(f32)
            sx = ((vx - f32(4.0)) + f32(0.5)).astype(f32)  # ref: start + offs[0]
            sy = ((vy - f32(4.0)) + f32(0.5)).astype(f32)
            fx = np.floor(sx).astype(np.int32)
            fy = np.floor(sy).astype(np.int32)
            lx = (sx - fx.astype(f32)).astype(f32)
            ly = (sy - fy.astype(f32)).astype(f32)
            fyb = fy - Y0[half]
            assert fyb.min() >= 0 and fyb.max() + 7 < BAND, (
                f"band overflow: fyb in [{fyb.min()}, {fyb.max()}]")

            pad = np.concatenate([np.arange(NV_CORE),
                                  np.full(VPAD - NV_CORE, NV_CORE - 1)])
            fxp, fybp = fx[pad], fyb[pad]
            lxp, lyp = lx[pad], ly[pad]

            idx = np.zeros((128, NR), np.int32)
            lxb = np.zeros((128, NR), f32)
            ayb = np.zeros((128, NR * 105), f32)
            for r in range(NR):
                blk = ayb[:, r * 105:(r + 1) * 105]
                for k in range(VR):
                    vcol = r * VR + k
                    for di in range(8):
                        idx[k * 8 + di, r] = (int(fybp[vcol]) + di) * 256 + int(fxp[vcol])
                    lxb[k * 8:(k + 1) * 8, r] = lxp[vcol]
                    lyv = lyp[vcol]
                    for i in range(7):
                        blk[k * 8 + i, k * 7 + i] += f32(1.0) - lyv
                        blk[k * 8 + i + 1, k * 7 + i] += lyv
                blk[120, :] = 1.0

            fband = feats_bf[b][:, Y0[half] * 256:(Y0[half] + BAND) * 256]

            in_maps.append({
                "fband": np.ascontiguousarray(fband),
                "wct": _bf16(wct),
                "ayb": _bf16(ayb),
                "idx": idx,
                "lxb": lxb,
                "bcrow": _bf16(bcrow),
                "onesb": _bf16(onesb),
                "ident": ident,
                "w1t": w1t.astype(f32),
                "b1a": b1a,
                "w2t": w2t.astype(f32),
                "b2a": b2a,
                "wht": wht.astype(f32),
                "bha": bha,
            })
    return in_maps, perms


def _build_graph():
    import concourse.bass as bass
    import concourse.mybir as mybir
    from concourse import bacc
    from concourse.tile import TileContext

    nc = bacc.Bacc("TRN2", num_devices=8)
    dt = mybir.dt
    f_ = dt.float32
    bf = dt.bfloat16

    fband = nc.dram_tensor("fband", [256, NP], bf, kind="ExternalInput")
    wct = nc.dram_tensor("wct", [128, 256], bf, kind="ExternalInput")
    ayb = nc.dram_tensor("ayb", [128, NR * 105], bf, kind="ExternalInput")
    idx = nc.dram_tensor("idx", [128, NR], dt.int32, kind="ExternalInput")
    lxb = nc.dram_tensor("lxb", [128, NR], f_, kind="ExternalInput")
    bcrow = nc.dram_tensor("bcrow", [8, 896], bf, kind="ExternalInput")
    onesb = nc.dram_tensor("onesb", [105, VR], bf, kind="ExternalInput")
    ident = nc.dram_tensor("ident", [128, 128], f_, kind="ExternalInput")
    w1t = nc.dram_tensor("w1t", [128, 256], f_, kind="ExternalInput")
    b1a = nc.dram_tensor("b1a", [128, 2], f_, kind="ExternalInput")
    w2t = nc.dram_tensor("w2t", [128, 256], f_, kind="ExternalInput")
    b2a = nc.dram_tensor("b2a", [128, 1], f_, kind="ExternalInput")
    wht = nc.dram_tensor("wht", [128, 4], f_, kind="ExternalInput")
    bha = nc.dram_tensor("bha", [4, 1], f_, kind="ExternalInput")
    out = nc.dram_tensor("out", [4, VPAD], f_, kind="ExternalOutput")
    table = nc.dram_tensor("gtab", [NP, 128], bf, kind="Internal")

    RELU = mybir.ActivationFunctionType.Relu
    IDEN = mybir.ActivationFunctionType.Identity
    ADD = mybir.AluOpType.add
    SUB = mybir.AluOpType.subtract
    MULT = mybir.AluOpType.mult

    with TileContext(nc) as tc:
        # ---------------- Phase 1: conv1x1 band -> pos-major table --------
        with (
            tc.tile_pool(name="cconst", bufs=1) as cpool,
            tc.tile_pool(name="fload", bufs=4) as fpool,
            tc.tile_pool(name="gstage", bufs=4) as gpool,
            tc.tile_pool(name="cpsum", bufs=4, space="PSUM") as cpsum,
        ):
            wct_sb = cpool.tile([128, 256], bf, tag="wct")
            nc.sync.dma_start(wct_sb[:, :], wct[:, :])
            for g in range(NG4):
                ft0 = fpool.tile([128, 512], bf, tag="f0")
                ft1 = fpool.tile([128, 512], bf, tag="f1")
                nc.sync.dma_start(ft0[:, :], fband[0:128, g * 512:(g + 1) * 512])
                nc.sync.dma_start(ft1[:, :], fband[128:256, g * 512:(g + 1) * 512])
                ps = cpsum.tile([128, 512], f_, tag="cps")
                for t in range(4):
                    nc.tensor.matmul(ps[:, t * 128:(t + 1) * 128],
                                     ft0[:, t * 128:(t + 1) * 128],
                                     wct_sb[:, 0:128], start=True, stop=False)
                    nc.tensor.matmul(ps[:, t * 128:(t + 1) * 128],
                                     ft1[:, t * 128:(t + 1) * 128],
                                     wct_sb[:, 128:256], start=False, stop=True)
                gt = gpool.tile([128, 512], bf, tag="gt")
                if g % 2 == 0:
                    nc.vector.tensor_copy(gt[:, :], ps[:, :])
                else:
                    nc.scalar.copy(gt[:, :], ps[:, :])
                for t in range(4):
                    nc.scalar.dma_start(
                        table[g * 512 + t * 128:g * 512 + (t + 1) * 128, :],
                        gt[:, t * 128:(t + 1) * 128])

        tc.strict_bb_all_engine_barrier()

        # ---------------- Phase 2: gather + interp + pool + MLP -----------
        with (
            tc.tile_pool(name="gconst", bufs=1) as ipool,
            tc.tile_pool(name="po", bufs=6) as popool,
            tc.tile_pool(name="interp", bufs=4) as xpool,
            tc.tile_pool(name="mpsum", bufs=2, space="PSUM") as mpsum,
            tc.tile_pool(name="ppsum", bufs=2, space="PSUM") as ppsum,
        ):
            idx_sb = ipool.tile([128, NR], dt.int32, tag="idx")
            nc.sync.dma_start(idx_sb[:, :], idx[:, :])
            lxb_sb = ipool.tile([128, NR], f_, tag="lxb")
            nc.sync.dma_start(lxb_sb[:, :], lxb[:, :])
            ayb_sb = ipool.tile([128, NR * 105], bf, tag="ayb")
            nc.sync.dma_start(ayb_sb[:, :], ayb[:, :])
            onesb_sb = ipool.tile([105, VR], bf, tag="onesb")
            nc.sync.dma_start(onesb_sb[:, :], onesb[:, :])
            ident_sb = ipool.tile([128, 128], f_, tag="ident")
            nc.sync.dma_start(ident_sb[:, :], ident[:, :])
            pooled = ipool.tile([128, VPAD], f_, tag="pooled")

            for r in range(NR):
                po = popool.tile([128, 1024], bf, tag="po")
                nc.gpsimd.indirect_dma_start(
                    out=po[0:120, :],
                    out_offset=None,
                    in_=table[:, :],
                    in_offset=bass.IndirectOffsetOnAxis(
                        ap=idx_sb[0:120, r:r + 1], axis=0),
                )
                d = xpool.tile([128, 896], bf, tag="d")
                nc.vector.tensor_tensor(
                    out=d[0:120, :], in0=po[0:120, 128:1024],
                    in1=po[0:120, 0:896], op=SUB)
                d2 = xpool.tile([128, 896], bf, tag="d2")
                nc.vector.tensor_scalar_mul(d2[0:120, :], d[0:120, :],
                                            lxb_sb[0:120, r:r + 1])
                tx = xpool.tile([128, 896], bf, tag="tx")
                nc.vector.tensor_tensor(
                    out=tx[0:120, :], in0=d2[0:120, :],
                    in1=po[0:120, 0:896], op=ADD)
                nc.sync.dma_start(tx[120:128, :], bcrow[:, :])

                comp = mpsum.tile([105, 896], f_, tag="comp")
                nc.tensor.matmul(comp[:, 0:512],
                                 ayb_sb[:, r * 105:(r + 1) * 105],
                                 tx[:, 0:512], start=True, stop=True)
                nc.tensor.matmul(comp[:, 512:896],
                                 ayb_sb[:, r * 105:(r + 1) * 105],
                                 tx[:, 512:896], start=True, stop=True)
                rl = xpool.tile([105, 896], bf, tag="rl")
                nc.scalar.activation(rl[:, :], comp[:, :], RELU)

                pj = ppsum.tile([15, 896], f_, tag="pj")
                nc.tensor.matmul(pj[:, 0:512], onesb_sb[:, :],
                                 rl[:, 0:512], start=True, stop=True)
                nc.tensor.matmul(pj[:, 512:896], onesb_sb[:, :],
                                 rl[:, 512:896], start=True, stop=True)
                pv = xpool.tile([15, 128], f_, tag="pv")
                nc.vector.tensor_reduce(
                    pv[:, :],
                    pj[:, :].rearrange("p (j o) -> p o j", o=128),
                    axis=mybir.AxisListType.X, op=ADD)
                pt = mpsum.tile([128, VR], f_, tag="comp")
                nc.tensor.transpose(pt[:, :], pv[:, :], ident_sb[0:15, 0:15])
                if r % 2 == 0:
                    nc.vector.tensor_copy(pooled[:, r * VR:(r + 1) * VR], pt[:, :])
                else:
                    nc.scalar.copy(pooled[:, r * VR:(r + 1) * VR], pt[:, :])

            # ---------------- Phase 3: MLP --------------------------------
            w1_sb = ipool.tile([128, 256], f_, tag="w1t")
            nc.sync.dma_start(w1_sb[:, :], w1t[:, :])
            b1_sb = ipool.tile([128, 2], f_, tag="b1a")
            nc.sync.dma_start(b1_sb[:, :], b1a[:, :])
            w2_sb = ipool.tile([128, 256], f_, tag="w2t")
            nc.sync.dma_start(w2_sb[:, :], w2t[:, :])
            b2_sb = ipool.tile([128, 1], f_, tag="b2a")
            nc.sync.dma_start(b2_sb[:, :], b2a[:, :])
            wh_sb = ipool.tile([128, 4], f_, tag="wht")
            nc.sync.dma_start(wh_sb[:, :], wht[:, :])
            bh_sb = ipool.tile([4, 1], f_, tag="bha")
            nc.sync.dma_start(bh_sb[:, :], bha[:, :])

            NSPL = [(0, 512), (512, VPAD)]
            h1 = ipool.tile([128, 2 * VPAD], f_, tag="h1")
            for hh in range(2):
                hp = mpsum.tile([128, VPAD], f_, tag="comp")
                for (a, e) in NSPL:
                    nc.tensor.matmul(hp[:, a:e],
                                     w1_sb[:, hh * 128:(hh + 1) * 128],
                                     pooled[:, a:e], start=True, stop=True)
                nc.scalar.activation(h1[:, hh * VPAD:(hh + 1) * VPAD], hp[:, :],
                                     RELU, bias=b1_sb[:, hh:hh + 1])
            h2 = ipool.tile([128, VPAD], f_, tag="h2")
            hp2 = mpsum.tile([128, VPAD], f_, tag="comp")
            for (a, e) in NSPL:
                nc.tensor.matmul(hp2[:, a:e], w2_sb[:, 0:128],
                                 h1[:, a:e], start=True, stop=False)
                nc.tensor.matmul(hp2[:, a:e], w2_sb[:, 128:256],
                                 h1[:, VPAD + a:VPAD + e], start=False, stop=True)
            nc.scalar.activation(h2[:, :], hp2[:, :], RELU, bias=b2_sb[:, 0:1])
            ho = ipool.tile([4, VPAD], f_, tag="ho")
            hpo = ppsum.tile([4, VPAD], f_, tag="pj")
            for (a, e) in NSPL:
                nc.tensor.matmul(hpo[:, a:e], wh_sb[:, :], h2[:, a:e],
                                 start=True, stop=True)
            nc.scalar.activation(ho[:, :], hpo[:, :], IDEN, bias=bh_sb[:, 0:1])
            nc.sync.dma_start(out[:, :], ho[:, :])

    nc.compile()
    return nc


_GRAPH_CACHE = {}


def kernel(features, vertices, Wc, bc, W1, b1, W2, b2, Wa, ba, Wr, br, Ww, bw):
    import sys
    if '/opt/trn_rl_repo' not in sys.path:
        sys.path.insert(0, '/opt/trn_rl_repo')
    from concourse import bass_utils

    in_maps, perms = _host_prep(features, vertices, Wc, bc, W1, b1, W2, b2,
                                Wa, ba, Wr, br, Ww, bw)
    if "nc" not in _GRAPH_CACHE:
        _GRAPH_CACHE["nc"] = _build_graph()
    nc = _GRAPH_CACHE["nc"]
    import os
    trace = bool(int(os.environ.get("KERNEL_TRACE", "0")))
    res = bass_utils.run_bass_kernel_spmd(nc, in_maps, core_ids=list(range(8)),
                                          trace=trace)
    kernel.last_results = res
    kernel.last_in_maps = in_maps

    f32 = np.float32
    d_angle = np.zeros((B, N, 2), f32)
    d_radius = np.zeros((B, N, 1), f32)
    d_width = np.zeros((B, N, 1), f32)
    for b in range(B):
        order = perms[b]
        for half in range(2):
            o = np.asarray(res.results[2 * b + half]["out"], f32)  # [4, 525]
            sel = order[half * NV_CORE:(half + 1) * NV_CORE]
            d_angle[b, sel, 0] = o[0, :NV_CORE]
            d_angle[b, sel, 1] = o[1, :NV_CORE]
            d_radius[b, sel, 0] = o[2, :NV_CORE]
            d_width[b, sel, 0] = o[3, :NV_CORE]
    return d_angle, d_radius, d_width


# revision 6
# speedup vs baseline: 1.6635x; 1.2711x over previous
"""TRN2 Bass kernel for nn_AdapterModule (ROI-align + conv1x1 + ReLU + avgpool + MLP).

Sharding: core = 2*b + half; 512 y-sorted vertices of batch b per core; each
core reads a 152-row y-band of its batch's features (bf16, host-cast).

Pipeline per core:
  1. conv1x1 (channels 256->128, weights pre-scaled 1/49) on the band,
     f-stationary matmuls -> pos-major bf16 table in DRAM [38912, 128].
  2. indirect-DMA gather: one 8-pixel row-run (2 KB) per (vertex, patch row),
     35 rounds x 15 vertices -> SBUF [partition=(v,di), free=(dj,o)].
  3. X-lerp on DVE (fused scalar_tensor_tensor, per-partition lx).
  4. Y-lerp + conv bias on PE: block-diag Ay matmul contracting partitions;
     a spare partition row injects the bias.
  5. ACT ReLU (PSUM->SBUF bf16); pool = ones-matmul over i (PE) + DVE reduce
     over j; PE transpose -> pooled [128 ch, vertex].
  6. MLP 128->256->128->4 on PE (3 heads fused into one [4,128] matmul).

The 49 ROI samples of a vertex share one fractional offset (sample spacing
== 1 px == bin size), so they form an 8x8 patch with a separable 2x2 stencil;
for vertices in [4, 252] clipping/validity never trigger.
"""

import numpy as np

ROI = 7
B, C, H, W, N = 4, 256, 256, 256, 1024
NV_CORE = 512       # vertices per core
VR = 15             # vertices per round
NR = 35             # rounds (35*15 = 525 >= 512)
VPAD = NR * VR      # 525
BAND = 152          # band rows per core
NP = BAND * 256     # band positions
Y0 = [0, 104]       # band start row for half 0 / 1
NG4 = NP // 512     # conv groups of 4 pos-tiles


def _bf16(x):
    import ml_dtypes
    return np.asarray(x, dtype=np.float32).astype(ml_dtypes.bfloat16)


def _host_prep(features, vertices, Wc, bc, W1, b1, W2, b2, Wa, ba, Wr, br, Ww, bw):
    f32 = np.float32
    feats_bf = _bf16(np.asarray(features, f32).reshape(B, C, H * W))

    wc49 = np.asarray(Wc, f32) / f32(49.0)                # [128, 256]
    bc49 = np.asarray(bc, f32) / f32(49.0)                # [128]
    wct = np.concatenate([wc49[:, 0:128].T, wc49[:, 128:256].T], axis=1)
    bcrow = np.tile(bc49[None, :], (8, 7)).astype(f32)    # [8, 896]

    onesb = np.zeros((105, VR), f32)
    for k in range(VR):
        onesb[k * 7:(k + 1) * 7, k] = 1.0

    w1 = np.asarray(W1, f32)
    w1t = np.concatenate([w1[:, 0:128][0:0]] if False else
                         [w1[0:128, :].T, w1[128:256, :].T], axis=1)  # [128,256]
    b1a = np.asarray(b1, f32).reshape(2, 128).T           # [128, 2]
    w2 = np.asarray(W2, f32)
    w2t = np.concatenate([w2[:, 0:128].T, w2[:, 128:256].T], axis=1)  # [128,256]
    b2a = np.asarray(b2, f32).reshape(128, 1)
    wh = np.concatenate([np.asarray(Wa, f32), np.asarray(Wr, f32),
                         np.asarray(Ww, f32)], axis=0)
    wht = np.ascontiguousarray(wh.T)                      # [128, 4]
    bha = np.concatenate([np.asarray(ba, f32), np.asarray(br, f32),
                          np.asarray(bw, f32)]).reshape(4, 1)
    ident = np.eye(128, dtype=f32)

    in_maps = []
    perms = []
    for b in range(B):
        v = np.asarray(vertices[b], f32)
        order = np.argsort(v[:, 1], kind="stable")
        perms.append(order)
        for half in range(2):
            sel = order[half * NV_CORE:(half + 1) * NV_CORE]
            vx = v[sel, 0].astype(f32)
            vy = v[sel, 1].astype(f32)
            sx = ((vx - f32(4.0)) + f32(0.5)).astype(f32)  # ref: start + offs[0]
            sy = ((vy - f32(4.0)) + f32(0.5)).astype(f32)
            fx = np.floor(sx).astype(np.int32)
            fy = np.floor(sy).astype(np.int32)
            lx = (sx - fx.astype(f32)).astype(f32)
            ly = (sy - fy.astype(f32)).astype(f32)
            fyb = fy - Y0[half]
            assert fyb.min() >= 0 and fyb.max() + 7 < BAND, (
                f"band overflow: fyb in [{fyb.min()}, {fyb.max()}]")

            pad = np.concatenate([np.arange(NV_CORE),
                                  np.full(VPAD - NV_CORE, NV_CORE - 1)])
            fxp, fybp = fx[pad], fyb[pad]
            lxp, lyp = lx[pad], ly[pad]

            idx = np.zeros((128, NR), np.int32)
            lxb = np.zeros((128, NR), f32)
            ayb = np.zeros((128, NR * 105), f32)
            for r in range(NR):
                blk = ayb[:, r * 105:(r + 1) * 105]
                for k in range(VR):
                    vcol = r * VR + k
                    for di in range(8):
                        idx[k * 8 + di, r] = (int(fybp[vcol]) + di) * 256 + int(fxp[vcol])
                    lxb[k * 8:(k + 1) * 8, r] = lxp[vcol]
                    lyv = lyp[vcol]
                    for i in range(7):
                        blk[k * 8 + i, k * 7 + i] += f32(1.0) - lyv
                        blk[k * 8 + i + 1, k * 7 + i] += lyv
                blk[120, :] = 1.0

            fband = feats_bf[b][:, Y0[half] * 256:(Y0[half] + BAND) * 256]

            in_maps.append({
                "fband": np.ascontiguousarray(fband),
                "wct": _bf16(wct),
                "ayb": _bf16(ayb),
                "idx": idx,
                "lxb": lxb,
                "bcrow": _bf16(bcrow),
                "onesb": _bf16(onesb),
                "ident": ident,
                "w1t": w1t.astype(f32),
                "b1a": b1a,
                "w2t": w2t.astype(f32),
                "b2a": b2a,
                "wht": wht.astype(f32),
                "bha": bha,
            })
    return in_maps, perms


def _build_graph():
    import concourse.bass as bass
    import concourse.mybir as mybir
    from concourse import bacc
    from concourse.tile import TileContext

    nc = bacc.Bacc("TRN2", num_devices=8)
    dt = mybir.dt
    f_ = dt.float32
    bf = dt.bfloat16

    fband = nc.dram_tensor("fband", [256, NP], bf, kind="ExternalInput")
    wct = nc.dram_tensor("wct", [128, 256], bf, kind="ExternalInput")
    ayb = nc.dram_tensor("ayb", [128, NR * 105], bf, kind="ExternalInput")
    idx = nc.dram_tensor("idx", [128, NR], dt.int32, kind="ExternalInput")
    lxb = nc.dram_tensor("lxb", [128, NR], f_, kind="ExternalInput")
    bcrow = nc.dram_tensor("bcrow", [8, 896], bf, kind="ExternalInput")
    onesb = nc.dram_tensor("onesb", [105, VR], bf, kind="ExternalInput")
    ident = nc.dram_tensor("ident", [128, 128], f_, kind="ExternalInput")
    w1t = nc.dram_tensor("w1t", [128, 256], f_, kind="ExternalInput")
    b1a = nc.dram_tensor("b1a", [128, 2], f_, kind="ExternalInput")
    w2t = nc.dram_tensor("w2t", [128, 256], f_, kind="ExternalInput")
    b2a = nc.dram_tensor("b2a", [128, 1], f_, kind="ExternalInput")
    wht = nc.dram_tensor("wht", [128, 4], f_, kind="ExternalInput")
    bha = nc.dram_tensor("bha", [4, 1], f_, kind="ExternalInput")
    out = nc.dram_tensor("out", [4, VPAD], f_, kind="ExternalOutput")
    table = nc.dram_tensor("gtab", [NP, 128], bf, kind="Internal")

    RELU = mybir.ActivationFunctionType.Relu
    IDEN = mybir.ActivationFunctionType.Identity
    ADD = mybir.AluOpType.add
    SUB = mybir.AluOpType.subtract
    MULT = mybir.AluOpType.mult

    with TileContext(nc) as tc:
        # ---------------- Phase 1: conv1x1 band -> pos-major table --------
        with (
            tc.tile_pool(name="cconst", bufs=1) as cpool,
            tc.tile_pool(name="fload", bufs=4) as fpool,
            tc.tile_pool(name="gstage", bufs=4) as gpool,
            tc.tile_pool(name="cpsum", bufs=4, space="PSUM") as cpsum,
        ):
            wct_sb = cpool.tile([128, 256], bf, tag="wct")
            nc.sync.dma_start(wct_sb[:, :], wct[:, :])
            NG8 = NP // 1024
            for g in range(NG8):
                ft0 = fpool.tile([128, 1024], bf, tag="f0")
                ft1 = fpool.tile([128, 1024], bf, tag="f1")
                nc.sync.dma_start(ft0[:, :], fband[0:128, g * 1024:(g + 1) * 1024])
                nc.sync.dma_start(ft1[:, :], fband[128:256, g * 1024:(g + 1) * 1024])
                for h in range(2):
                    ps = cpsum.tile([128, 512], f_, tag="cps")
                    for t in range(4):
                        c = h * 512 + t * 128
                        nc.tensor.matmul(ps[:, t * 128:(t + 1) * 128],
                                         ft0[:, c:c + 128],
                                         wct_sb[:, 0:128], start=True, stop=False)
                        nc.tensor.matmul(ps[:, t * 128:(t + 1) * 128],
                                         ft1[:, c:c + 128],
                                         wct_sb[:, 128:256], start=False, stop=True)
                    gt = gpool.tile([128, 512], bf, tag="gt")
                    if h == 0:
                        nc.vector.tensor_copy(gt[:, :], ps[:, :])
                    else:
                        nc.scalar.copy(gt[:, :], ps[:, :])
                    eng = (nc.gpsimd, nc.scalar, nc.sync)[(2 * g + h) % 3]
                    for t in range(4):
                        base = g * 1024 + h * 512 + t * 128
                        eng.dma_start(table[base:base + 128, :],
                                      gt[:, t * 128:(t + 1) * 128])

        tc.strict_bb_all_engine_barrier()

        # ---------------- Phase 2: gather + interp + pool + MLP -----------
        with (
            tc.tile_pool(name="gconst", bufs=1) as ipool,
            tc.tile_pool(name="po", bufs=6) as popool,
            tc.tile_pool(name="interp", bufs=4) as xpool,
            tc.tile_pool(name="mpsum", bufs=2, space="PSUM") as mpsum,
            tc.tile_pool(name="ppsum", bufs=2, space="PSUM") as ppsum,
        ):
            idx_sb = ipool.tile([128, NR], dt.int32, tag="idx")
            nc.sync.dma_start(idx_sb[:, :], idx[:, :])
            lxb_sb = ipool.tile([128, NR], f_, tag="lxb")
            nc.sync.dma_start(lxb_sb[:, :], lxb[:, :])
            ayb_sb = ipool.tile([128, NR * 105], bf, tag="ayb")
            nc.sync.dma_start(ayb_sb[:, :], ayb[:, :])
            onesb_sb = ipool.tile([105, VR], bf, tag="onesb")
            nc.sync.dma_start(onesb_sb[:, :], onesb[:, :])
            ident_sb = ipool.tile([128, 128], f_, tag="ident")
            nc.sync.dma_start(ident_sb[:, :], ident[:, :])
            pooled = ipool.tile([128, VPAD], f_, tag="pooled")

            for r in range(NR):
                po = popool.tile([128, 1024], bf, tag="po")
                nc.gpsimd.indirect_dma_start(
                    out=po[0:120, :],
                    out_offset=None,
                    in_=table[:, :],
                    in_offset=bass.IndirectOffsetOnAxis(
                        ap=idx_sb[0:120, r:r + 1], axis=0),
                )
                d = xpool.tile([128, 896], bf, tag="d")
                nc.vector.tensor_tensor(
                    out=d[0:120, :], in0=po[0:120, 128:1024],
                    in1=po[0:120, 0:896], op=SUB)
                d2 = xpool.tile([128, 896], bf, tag="d2")
                nc.vector.tensor_scalar_mul(d2[0:120, :], d[0:120, :],
                                            lxb_sb[0:120, r:r + 1])
                tx = xpool.tile([128, 896], bf, tag="tx")
                nc.vector.tensor_tensor(
                    out=tx[0:120, :], in0=d2[0:120, :],
                    in1=po[0:120, 0:896], op=ADD)
                nc.sync.dma_start(tx[120:128, :], bcrow[:, :])

                comp = mpsum.tile([105, 896], f_, tag="comp")
                nc.tensor.matmul(comp[:, 0:512],
                                 ayb_sb[:, r * 105:(r + 1) * 105],
                                 tx[:, 0:512], start=True, stop=True)
                nc.tensor.matmul(comp[:, 512:896],
                                 ayb_sb[:, r * 105:(r + 1) * 105],
                                 tx[:, 512:896], start=True, stop=True)
                rl = xpool.tile([105, 896], bf, tag="rl")
                nc.scalar.activation(rl[:, :], comp[:, :], RELU)

                pj = ppsum.tile([15, 896], f_, tag="pj")
                nc.tensor.matmul(pj[:, 0:512], onesb_sb[:, :],
                                 rl[:, 0:512], start=True, stop=True)
                nc.tensor.matmul(pj[:, 512:896], onesb_sb[:, :],
                                 rl[:, 512:896], start=True, stop=True)
                pv = xpool.tile([15, 128], f_, tag="pv")
                nc.vector.tensor_reduce(
                    pv[:, :],
                    pj[:, :].rearrange("p (j o) -> p o j", o=128),
                    axis=mybir.AxisListType.X, op=ADD)
                pt = mpsum.tile([128, VR], f_, tag="comp")
                nc.tensor.transpose(pt[:, :], pv[:, :], ident_sb[0:15, 0:15])
                if r % 2 == 0:
                    nc.vector.tensor_copy(pooled[:, r * VR:(r + 1) * VR], pt[:, :])
                else:
                    nc.scalar.copy(pooled[:, r * VR:(r + 1) * VR], pt[:, :])

            # ---------------- Phase 3: MLP --------------------------------
            w1_sb = ipool.tile([128, 256], f_, tag="w1t")
            nc.sync.dma_start(w1_sb[:, :], w1t[:, :])
            b1_sb = ipool.tile([128, 2], f_, tag="b1a")
            nc.sync.dma_start(b1_sb[:, :], b1a[:, :])
            w2_sb = ipool.tile([128, 256], f_, tag="w2t")
            nc.sync.dma_start(w2_sb[:, :], w2t[:, :])
            b2_sb = ipool.tile([128, 1], f_, tag="b2a")
            nc.sync.dma_start(b2_sb[:, :], b2a[:, :])
            wh_sb = ipool.tile([128, 4], f_, tag="wht")
            nc.sync.dma_start(wh_sb[:, :], wht[:, :])
            bh_sb = ipool.tile([4, 1], f_, tag="bha")
            nc.sync.dma_start(bh_sb[:, :], bha[:, :])

            NSPL = [(0, 512), (512, VPAD)]
            h1 = ipool.tile([128, 2 * VPAD], f_, tag="h1")
            for hh in range(2):
                hp = mpsum.tile([128, VPAD], f_, tag="comp")
                for (a, e) in NSPL:
                    nc.tensor.matmul(hp[:, a:e],
                                     w1_sb[:, hh * 128:(hh + 1) * 128],
                                     pooled[:, a:e], start=True, stop=True)
                nc.scalar.activation(h1[:, hh * VPAD:(hh + 1) * VPAD], hp[:, :],
                                     RELU, bias=b1_sb[:, hh:hh + 1])
            h2 = ipool.tile([128, VPAD], f_, tag="h2")
            hp2 = mpsum.tile([128, VPAD], f_, tag="comp")
            for (a, e) in NSPL:
                nc.tensor.matmul(hp2[:, a:e], w2_sb[:, 0:128],
                                 h1[:, a:e], start=True, stop=False)
                nc.tensor.matmul(hp2[:, a:e], w2_sb[:, 128:256],
                                 h1[:, VPAD + a:VPAD + e], start=False, stop=True)
            nc.scalar.activation(h2[:, :], hp2[:, :], RELU, bias=b2_sb[:, 0:1])
            ho = ipool.tile([4, VPAD], f_, tag="ho")
            hpo = ppsum.tile([4, VPAD], f_, tag="pj")
            for (a, e) in NSPL:
                nc.tensor.matmul(hpo[:, a:e], wh_sb[:, :], h2[:, a:e],
                                 start=True, stop=True)
            nc.scalar.activation(ho[:, :], hpo[:, :], IDEN, bias=bh_sb[:, 0:1])
            nc.sync.dma_start(out[:, :], ho[:, :])

    nc.compile()
    return nc


_GRAPH_CACHE = {}


def kernel(features, vertices, Wc, bc, W1, b1, W2, b2, Wa, ba, Wr, br, Ww, bw):
    import sys
    if '/opt/trn_rl_repo' not in sys.path:
        sys.path.insert(0, '/opt/trn_rl_repo')
    from concourse import bass_utils

    in_maps, perms = _host_prep(features, vertices, Wc, bc, W1, b1, W2, b2,
                                Wa, ba, Wr, br, Ww, bw)
    if "nc" not in _GRAPH_CACHE:
        _GRAPH_CACHE["nc"] = _build_graph()
    nc = _GRAPH_CACHE["nc"]
    import os
    trace = bool(int(os.environ.get("KERNEL_TRACE", "0")))
    res = bass_utils.run_bass_kernel_spmd(nc, in_maps, core_ids=list(range(8)),
                                          trace=trace)
    kernel.last_results = res
    kernel.last_in_maps = in_maps

    f32 = np.float32
    d_angle = np.zeros((B, N, 2), f32)
    d_radius = np.zeros((B, N, 1), f32)
    d_width = np.zeros((B, N, 1), f32)
    for b in range(B):
        order = perms[b]
        for half in range(2):
            o = np.asarray(res.results[2 * b + half]["out"], f32)  # [4, 525]
            sel = order[half * NV_CORE:(half + 1) * NV_CORE]
            d_angle[b, sel, 0] = o[0, :NV_CORE]
            d_angle[b, sel, 1] = o[1, :NV_CORE]
            d_radius[b, sel, 0] = o[2, :NV_CORE]
            d_width[b, sel, 0] = o[3, :NV_CORE]
    return d_angle, d_radius, d_width
